# revision 25
# baseline (speedup 1.0000x reference)
"""Trainium2 Bass kernel for nn_ETypePromptModel: logits = einsum('bpd,cpd->bc').

Equivalent to X @ W.T with X=[B, L*D]=[16384, 256], W=[C, L*D]=[4096, 256].
Data-parallel over B across 8 NeuronCores; label2embed replicated.

bf16 pipeline (tolerance 2e-2; bf16 lands ~0.34%, fp8 measured 3.8% - dead):
  - Host: cast to bf16, pre-transpose to K-major, and pack so bulk DMAs
    land 8KB-contiguous per partition: the HWDGE generates descriptors at
    ~18ns each (so 4KB descriptors cap a ring at ~227 GB/s) and one
    straggler SDMA engine can lag ~2.3us behind the other 15, so loads
    are ordered/sized so every tile lands well before first use:
      W n-half A, x0 (X m-tiles 0..XD-1 duplicate), W n-half B, full X.
  - Stream is phase-reordered around the loads: n-half-A of m-tiles
    0..XD-1 (fed from x0) runs before any B-half; full X is only needed
    from m-tile XD, several us after it lands.
  - bf16 warmup matmuls on a memset tile bridge issue->data so the HAM
    clock gate reaches 8/8 right as the real stream starts.
  - Per (m-tile, n-half): 4 chunk matmuls of 512 cols x 2 k-passes
    accumulating in fp32 PSUM; 4 two-bank PSUM pair-tiles rotate; steady
    state issues one matmul per 216ns with LDWEIGHTS hidden.
  - PSUM -> SBUF drains as one 1024-wide cast per engine per half
    (Vector even pair, Scalar odd pair); per m-tile one full-row 1MB
    output DMA (8KB descriptors) on the sync ring.
  - Host: upcast gathered bf16 output to fp32.

PE stream floor: 16 mt x 2 k x 4096 cols = 131072 cycles @ 2.4 GHz = 55 us.
Fixed overheads outside the stream: ~5.5us DMA issue+latency+lead-in,
~4.5us output tail, ~8.5us DMA-receipt + TileContext semaphore epilogue.
"""

import sys

import ml_dtypes
import numpy as np

sys.path.insert(0, "/opt/trn_rl_repo")

B, C, L, D = 16384, 4096, 2, 128
K = L * D  # 256 contraction
N_CORES = 8
B_LOC = B // N_CORES  # 2048
P = 128
N_TILE = 512  # moving free dim per matmul
M_TILES = B_LOC // P  # 16
KT = K // P  # 2 k-tiles
WH = C // 2  # 2048: w n-half width
NH = WH // N_TILE  # 4 chunks per half
XD = 3  # m-tiles covered by the duplicated X head

N_PAIRS = 4  # two-bank PSUM pair tiles (8 banks total)
N_OSB = 6
N_WARM = 11  # warmup matmuls (~427ns each cold) bridging the load window

_CACHE = {}
PROFILE = False
TRACE_ALL_CORES = False
LAST_RESULT = None


def _build():
    import concourse.mybir as mybir
    import concourse.tile as tile
    from concourse import bacc

    f32 = mybir.dt.float32
    bf16 = mybir.dt.bfloat16

    nc = bacc.Bacc(
        "TRN2",
        target_bir_lowering=False,
        debug=False,
        enable_asserts=False,
        num_devices=N_CORES,
    )

    x0_dram = nc.dram_tensor("x0", [P, KT, XD * P], bf16, kind="ExternalInput").ap()
    x_dram = nc.dram_tensor("xt", [P, KT, B_LOC], bf16, kind="ExternalInput").ap()
    w_dram = nc.dram_tensor("wt", [2, P, KT, WH], bf16, kind="ExternalInput").ap()
    out_dram = nc.dram_tensor("out", [B_LOC, C], bf16, kind="ExternalOutput").ap()

    with tile.TileContext(nc) as tc:
        with (
            tc.tile_pool(name="const", bufs=1) as const_pool,
            tc.tile_pool(name="big", bufs=1) as big_pool,
            tc.tile_pool(name="osb", bufs=1) as out_pool,
            tc.tile_pool(name="psm", bufs=1, space="PSUM") as psum_mm,
        ):
            # ---- input loads: consumption order, single sync HWDGE ring ----
            x0 = big_pool.tile([P, KT, XD * P], bf16, name="x0")
            xk = big_pool.tile([P, KT, B_LOC], bf16, name="xk")
            wk = [big_pool.tile([P, KT, WH], bf16, name=f"wk{h}") for h in range(2)]
            nc.sync.dma_start(wk[0], w_dram[0])
            nc.sync.dma_start(x0, x0_dram)
            nc.sync.dma_start(wk[1], w_dram[1])
            nc.sync.dma_start(xk, x_dram)

            # ---- PE warmup on a memset tile (HAM un-throttles ~3.4us into
            # the burst, right as the real stream starts) ----
            warm_sb = const_pool.tile([P, P + N_TILE], bf16, name="warm_sb")
            nc.vector.memset(warm_sb, 0.0)

            # ---- manually reused buffers ----
            pairs = [
                psum_mm.tile([P, 2, N_TILE], f32, name=f"pp{i}") for i in range(N_PAIRS)
            ]
            osb = [out_pool.tile([P, C], bf16, name=f"osb{i}") for i in range(N_OSB)]

            for _ in range(N_WARM):
                nc.tensor.matmul(
                    pairs[N_PAIRS - 1][:, 1, :],
                    warm_sb[:, :P],
                    warm_sb[:, P:],
                    start=True,
                    stop=True,
                )

            # ---- main stream, phase-reordered around the input loads ----
            order = [(mt, 0) for mt in range(XD)] + [(mt, 1) for mt in range(XD)]
            for mt in range(XD, M_TILES):
                order += [(mt, 0), (mt, 1)]

            pc = 0
            for mt, h in order:
                out_sb = osb[mt % N_OSB]
                prs = [pairs[(pc + a) % N_PAIRS] for a in range(2)]
                banks = [prs[j // 2][:, j % 2, :] for j in range(NH)]
                pc += 2
                xsrc = x0 if (mt < XD and h == 0) else xk
                for k in range(KT):
                    xs = xsrc[:, k, mt * P : (mt + 1) * P]
                    for j in range(NH):
                        nc.tensor.matmul(
                            banks[j],
                            xs,
                            wk[h][:, k, j * N_TILE : (j + 1) * N_TILE],
                            start=(k == 0),
                            stop=(k == KT - 1),
                        )
                # one 1024-wide PSUM->SBUF cast per engine per half
                off = h * WH
                nc.vector.tensor_copy(
                    out=out_sb[:, off : off + 2 * N_TILE],
                    in_=prs[0].rearrange("p a b -> p (a b)"),
                )
                nc.scalar.copy(
                    out_sb[:, off + 2 * N_TILE : off + 4 * N_TILE],
                    prs[1].rearrange("p a b -> p (a b)"),
                )
                if h == 1:
                    nc.sync.dma_start(out_dram[mt * P : (mt + 1) * P, :], out_sb)

    nc.compile()
    return nc


def kernel(batchs, label2embed):
    global LAST_RESULT
    from concourse.bass_utils import run_bass_kernel_spmd

    if "nc" not in _CACHE:
        _CACHE["nc"] = _build()
    nc = _CACHE["nc"]

    assert batchs.shape == (B, L, D) and label2embed.shape == (C, L, D)
    bf16 = ml_dtypes.bfloat16
    # K-major bf16, packed for 8KB/partition DMA rows:
    #   xt: [P, KT, B] (row p = k0-row-p ++ k1-row-p)
    #   wt: [2, P, KT, WH] (half h, row p = k0-cols ++ k1-cols)
    xtf = batchs.reshape(B, K).astype(bf16).T.reshape(KT, P, B)  # [KT, P, B]
    wtf = label2embed.reshape(C, K).astype(bf16).T.reshape(KT, P, C)
    xt = np.ascontiguousarray(xtf.transpose(1, 0, 2))  # [P, KT, B]
    wt = np.ascontiguousarray(
        np.stack(
            [wtf[:, :, h * WH : (h + 1) * WH].transpose(1, 0, 2) for h in range(2)]
        )
    )  # [2, P, KT, WH]
    in_maps = [
        {
            "x0": np.ascontiguousarray(xt[:, :, c * B_LOC : c * B_LOC + XD * P]),
            "xt": np.ascontiguousarray(xt[:, :, c * B_LOC : (c + 1) * B_LOC]),
            "wt": wt,
        }
        for c in range(N_CORES)
    ]
    res = run_bass_kernel_spmd(
        nc,
        in_maps,
        core_ids=list(range(N_CORES)),
        trace=PROFILE,
        trace_cores=list(range(N_CORES)) if (PROFILE and TRACE_ALL_CORES) else None,
    )
    LAST_RESULT = res
    return np.concatenate([r["out"] for r in res.results], axis=0).astype(np.float32)


# revision 27
# speedup vs baseline: 1.0066x; 1.0066x over previous
"""Trainium2 Bass kernel for nn_ETypePromptModel: logits = einsum('bpd,cpd->bc').

Equivalent to X @ W.T with X=[B, L*D]=[16384, 256], W=[C, L*D]=[4096, 256].
Data-parallel over B across 8 NeuronCores; label2embed replicated.

bf16 pipeline (tolerance 2e-2; bf16 lands ~0.34%, fp8 measured 3.8% - dead):
  - Host: cast to bf16, pre-transpose to K-major, and pack so bulk DMAs
    land 8KB-contiguous per partition: the HWDGE generates descriptors at
    ~18ns each (so 4KB descriptors cap a ring at ~227 GB/s) and one
    straggler SDMA engine can lag ~2.3us behind the other 15, so loads
    are ordered/sized so every tile lands well before first use:
      W n-half A, x0 (X m-tiles 0..XD-1 duplicate), W n-half B, full X.
  - Stream is phase-reordered around the loads: n-half-A of m-tiles
    0..XD-1 (fed from x0) runs before any B-half; full X is only needed
    from m-tile XD, several us after it lands.
  - bf16 warmup matmuls on a memset tile bridge issue->data so the HAM
    clock gate reaches 8/8 right as the real stream starts.
  - Per (m-tile, n-half): 4 chunk matmuls of 512 cols x 2 k-passes
    accumulating in fp32 PSUM; 4 two-bank PSUM pair-tiles rotate; steady
    state issues one matmul per 216ns with LDWEIGHTS hidden.
  - PSUM -> SBUF drains as one 1024-wide cast per engine per half
    (Vector even pair, Scalar odd pair); per m-tile one full-row 1MB
    output DMA (8KB descriptors) on the sync ring.
  - Host: upcast gathered bf16 output to fp32.

PE stream floor: 16 mt x 2 k x 4096 cols = 131072 cycles @ 2.4 GHz = 55 us.
Fixed overheads outside the stream: ~5.5us DMA issue+latency+lead-in,
~4.5us output tail, ~8.5us DMA-receipt + TileContext semaphore epilogue.
"""

import sys

import ml_dtypes
import numpy as np

sys.path.insert(0, "/opt/trn_rl_repo")

B, C, L, D = 16384, 4096, 2, 128
K = L * D  # 256 contraction
N_CORES = 8
B_LOC = B // N_CORES  # 2048
P = 128
N_TILE = 512  # moving free dim per matmul
M_TILES = B_LOC // P  # 16
KT = K // P  # 2 k-tiles
WH = C // 2  # 2048: w n-half width
NH = WH // N_TILE  # 4 chunks per half
XD = 3  # m-tiles covered by the duplicated X head

N_PAIRS = 4  # two-bank PSUM pair tiles (8 banks total)
N_OSB = 6
N_WARM = 11  # warmup matmuls (~427ns each cold) bridging the load window

_CACHE = {}
PROFILE = False
TRACE_ALL_CORES = False
LAST_RESULT = None


def _build():
    import concourse.mybir as mybir
    import concourse.tile as tile
    from concourse import bacc

    f32 = mybir.dt.float32
    bf16 = mybir.dt.bfloat16

    nc = bacc.Bacc(
        "TRN2",
        target_bir_lowering=False,
        debug=False,
        enable_asserts=False,
        num_devices=N_CORES,
    )

    x0_dram = nc.dram_tensor("x0", [P, KT, XD * P], bf16, kind="ExternalInput").ap()
    x_dram = nc.dram_tensor("xt", [P, KT, B_LOC], bf16, kind="ExternalInput").ap()
    w_dram = nc.dram_tensor("wt", [2, P, KT, WH], bf16, kind="ExternalInput").ap()
    out_dram = nc.dram_tensor("out", [B_LOC, C], bf16, kind="ExternalOutput").ap()

    with tile.TileContext(nc) as tc:
        with (
            tc.tile_pool(name="const", bufs=1) as const_pool,
            tc.tile_pool(name="big", bufs=1) as big_pool,
            tc.tile_pool(name="osb", bufs=1) as out_pool,
            tc.tile_pool(name="psm", bufs=1, space="PSUM") as psum_mm,
        ):
            # ---- input loads: consumption order, single sync HWDGE ring ----
            x0 = big_pool.tile([P, KT, XD * P], bf16, name="x0")
            xk = big_pool.tile([P, KT, B_LOC], bf16, name="xk")
            wk = [big_pool.tile([P, KT, WH], bf16, name=f"wk{h}") for h in range(2)]
            nc.sync.dma_start(wk[0], w_dram[0])
            nc.sync.dma_start(x0, x0_dram)
            nc.sync.dma_start(wk[1], w_dram[1])
            nc.sync.dma_start(xk, x_dram)

            # ---- PE warmup on a memset tile (HAM un-throttles ~3.4us into
            # the burst, right as the real stream starts) ----
            warm_sb = const_pool.tile([P, P + N_TILE], bf16, name="warm_sb")
            nc.vector.memset(warm_sb, 0.0)

            # ---- manually reused buffers ----
            pairs = [
                psum_mm.tile([P, 2, N_TILE], f32, name=f"pp{i}") for i in range(N_PAIRS)
            ]
            osb = [out_pool.tile([P, C], bf16, name=f"osb{i}") for i in range(N_OSB)]

            for _ in range(N_WARM):
                nc.tensor.matmul(
                    pairs[N_PAIRS - 1][:, 1, :],
                    warm_sb[:, :P],
                    warm_sb[:, P:],
                    start=True,
                    stop=True,
                )

            # ---- main stream, phase-reordered around the input loads ----
            order = [(mt, 0) for mt in range(XD)] + [(mt, 1) for mt in range(XD)]
            for mt in range(XD, M_TILES):
                order += [(mt, 0), (mt, 1)]

            pc = 0
            for mt, h in order:
                out_sb = osb[mt % N_OSB]
                prs = [pairs[(pc + a) % N_PAIRS] for a in range(2)]
                banks = [prs[j // 2][:, j % 2, :] for j in range(NH)]
                pc += 2
                xsrc = x0 if (mt < XD and h == 0) else xk
                for k in range(KT):
                    xs = xsrc[:, k, mt * P : (mt + 1) * P]
                    for j in range(NH):
                        nc.tensor.matmul(
                            banks[j],
                            xs,
                            wk[h][:, k, j * N_TILE : (j + 1) * N_TILE],
                            start=(k == 0),
                            stop=(k == KT - 1),
                        )
                # one 1024-wide PSUM->SBUF cast per engine per half
                off = h * WH
                nc.vector.tensor_copy(
                    out=out_sb[:, off : off + 2 * N_TILE],
                    in_=prs[0].rearrange("p a b -> p (a b)"),
                )
                nc.scalar.copy(
                    out_sb[:, off + 2 * N_TILE : off + 4 * N_TILE],
                    prs[1].rearrange("p a b -> p (a b)"),
                )
                if h == 1:
                    nc.sync.dma_start(out_dram[mt * P : (mt + 1) * P, :], out_sb)

    nc.compile()
    return nc


def kernel(batchs, label2embed):
    global LAST_RESULT
    from concourse.bass_utils import run_bass_kernel_spmd

    if "nc" not in _CACHE:
        _CACHE["nc"] = _build()
    nc = _CACHE["nc"]

    assert batchs.shape == (B, L, D) and label2embed.shape == (C, L, D)
    bf16 = ml_dtypes.bfloat16
    # K-major bf16, packed for 8KB/partition DMA rows:
    #   xt: [P, KT, B] (row p = k0-row-p ++ k1-row-p)
    #   wt: [2, P, KT, WH] (half h, row p = k0-cols ++ k1-cols)
    xtf = batchs.reshape(B, K).astype(bf16).T.reshape(KT, P, B)  # [KT, P, B]
    wtf = label2embed.reshape(C, K).astype(bf16).T.reshape(KT, P, C)
    xt = np.ascontiguousarray(xtf.transpose(1, 0, 2))  # [P, KT, B]
    wt = np.ascontiguousarray(
        np.stack(
            [wtf[:, :, h * WH : (h + 1) * WH].transpose(1, 0, 2) for h in range(2)]
        )
    )  # [2, P, KT, WH]
    in_maps = [
        {
            "x0": np.ascontiguousarray(xt[:, :, c * B_LOC : c * B_LOC + XD * P]),
            "xt": np.ascontiguousarray(xt[:, :, c * B_LOC : (c + 1) * B_LOC]),
            "wt": wt,
        }
        for c in range(N_CORES)
    ]
    res = run_bass_kernel_spmd(
        nc,
        in_maps,
        core_ids=list(range(N_CORES)),
        trace=PROFILE,
        trace_cores=list(range(N_CORES)) if (PROFILE and TRACE_ALL_CORES) else None,
    )
    LAST_RESULT = res
    return np.concatenate([r["out"] for r in res.results], axis=0).astype(np.float32)


# revision 29
# speedup vs baseline: 1.0105x; 1.0039x over previous
"""Trainium2 Bass kernel for nn_ETypePromptModel: logits = einsum('bpd,cpd->bc').

Equivalent to X @ W.T with X=[B, L*D]=[16384, 256], W=[C, L*D]=[4096, 256].
Data-parallel over B across 8 NeuronCores; label2embed replicated.

bf16 pipeline (tolerance 2e-2; bf16 lands ~0.34%, fp8 measured 3.8% - dead):
  - Host: cast to bf16, pre-transpose to K-major, and pack so bulk DMAs
    land 8KB-contiguous per partition: the HWDGE generates descriptors at
    ~18ns each (so 4KB descriptors cap a ring at ~227 GB/s) and one
    straggler SDMA engine can lag ~2.3us behind the other 15, so loads
    are ordered/sized so every tile lands well before first use:
      W n-half A, x0 (X m-tiles 0..XD-1 duplicate), W n-half B, full X.
  - Stream is phase-reordered around the loads: n-half-A of m-tiles
    0..XD-1 (fed from x0) runs before any B-half; full X is only needed
    from m-tile XD, several us after it lands.
  - bf16 warmup matmuls on a memset tile bridge issue->data so the HAM
    clock gate reaches 8/8 right as the real stream starts.
  - Per (m-tile, n-half): 4 chunk matmuls of 512 cols x 2 k-passes
    accumulating in fp32 PSUM; 4 two-bank PSUM pair-tiles rotate; steady
    state issues one matmul per 216ns with LDWEIGHTS hidden.
  - PSUM -> SBUF drains as one 1024-wide cast per engine per half
    (Vector even pair, Scalar odd pair); per m-tile one full-row 1MB
    output DMA (8KB descriptors) on the sync ring.
  - Host: upcast gathered bf16 output to fp32.

PE stream floor: 16 mt x 2 k x 4096 cols = 131072 cycles @ 2.4 GHz = 55 us.
Fixed overheads outside the stream: ~5.5us DMA issue+latency+lead-in,
~4.5us output tail, ~8.5us DMA-receipt + TileContext semaphore epilogue.
"""

import sys

import ml_dtypes
import numpy as np

sys.path.insert(0, "/opt/trn_rl_repo")

B, C, L, D = 16384, 4096, 2, 128
K = L * D  # 256 contraction
N_CORES = 8
B_LOC = B // N_CORES  # 2048
P = 128
N_TILE = 512  # moving free dim per matmul
M_TILES = B_LOC // P  # 16
KT = K // P  # 2 k-tiles
WH = C // 2  # 2048: w n-half width
NH = WH // N_TILE  # 4 chunks per half
XD = 3  # m-tiles covered by the duplicated X head

N_PAIRS = 4  # two-bank PSUM pair tiles (8 banks total)
N_OSB = 6
N_WARM = 11  # warmup matmuls (~427ns each cold) bridging the load window

_CACHE = {}
PROFILE = False
TRACE_ALL_CORES = False
LAST_RESULT = None


def _build():
    import concourse.mybir as mybir
    import concourse.tile as tile
    from concourse import bacc

    f32 = mybir.dt.float32
    bf16 = mybir.dt.bfloat16

    nc = bacc.Bacc(
        "TRN2",
        target_bir_lowering=False,
        debug=False,
        enable_asserts=False,
        num_devices=N_CORES,
    )

    x0_dram = nc.dram_tensor("x0", [P, KT, XD * P], bf16, kind="ExternalInput").ap()
    x_dram = nc.dram_tensor("xt", [P, KT, B_LOC], bf16, kind="ExternalInput").ap()
    w_dram = nc.dram_tensor("wt", [2, P, KT, WH], bf16, kind="ExternalInput").ap()
    out_dram = nc.dram_tensor("out", [B_LOC, C], bf16, kind="ExternalOutput").ap()

    with tile.TileContext(nc) as tc:
        with (
            tc.tile_pool(name="const", bufs=1) as const_pool,
            tc.tile_pool(name="big", bufs=1) as big_pool,
            tc.tile_pool(name="osb", bufs=1) as out_pool,
            tc.tile_pool(name="psm", bufs=1, space="PSUM") as psum_mm,
        ):
            # ---- input loads: consumption order, single sync HWDGE ring ----
            x0 = big_pool.tile([P, KT, XD * P], bf16, name="x0")
            xk = big_pool.tile([P, KT, B_LOC], bf16, name="xk")
            wk = [big_pool.tile([P, KT, WH], bf16, name=f"wk{h}") for h in range(2)]
            nc.sync.dma_start(wk[0], w_dram[0])
            nc.sync.dma_start(x0, x0_dram)
            nc.sync.dma_start(wk[1], w_dram[1])
            nc.sync.dma_start(xk, x_dram)

            # ---- PE warmup on a memset tile (HAM un-throttles ~3.4us into
            # the burst, right as the real stream starts) ----
            warm_sb = const_pool.tile([P, P + N_TILE], bf16, name="warm_sb")
            nc.vector.memset(warm_sb, 0.0)

            # ---- manually reused buffers ----
            pairs = [
                psum_mm.tile([P, 2, N_TILE], f32, name=f"pp{i}") for i in range(N_PAIRS)
            ]
            osb = [out_pool.tile([P, C], bf16, name=f"osb{i}") for i in range(N_OSB)]

            for _ in range(N_WARM):
                nc.tensor.matmul(
                    pairs[N_PAIRS - 1][:, 1, :],
                    warm_sb[:, :P],
                    warm_sb[:, P:],
                    start=True,
                    stop=True,
                )

            # ---- main stream, phase-reordered around the input loads ----
            order = [(mt, 0) for mt in range(XD)] + [(mt, 1) for mt in range(XD)]
            for mt in range(XD, M_TILES):
                order += [(mt, 0), (mt, 1)]

            pc = 0
            for mt, h in order:
                out_sb = osb[mt % N_OSB]
                prs = [pairs[(pc + a) % N_PAIRS] for a in range(2)]
                banks = [prs[j // 2][:, j % 2, :] for j in range(NH)]
                pc += 2
                xsrc = x0 if (mt < XD and h == 0) else xk
                for k in range(KT):
                    xs = xsrc[:, k, mt * P : (mt + 1) * P]
                    for j in range(NH):
                        nc.tensor.matmul(
                            banks[j],
                            xs,
                            wk[h][:, k, j * N_TILE : (j + 1) * N_TILE],
                            start=(k == 0),
                            stop=(k == KT - 1),
                        )
                # one 1024-wide PSUM->SBUF cast per engine per half
                off = h * WH
                nc.vector.tensor_copy(
                    out=out_sb[:, off : off + 2 * N_TILE],
                    in_=prs[0].rearrange("p a b -> p (a b)"),
                )
                nc.scalar.copy(
                    out_sb[:, off + 2 * N_TILE : off + 4 * N_TILE],
                    prs[1].rearrange("p a b -> p (a b)"),
                )
                if h == 1:
                    nc.sync.dma_start(out_dram[mt * P : (mt + 1) * P, :], out_sb)

    nc.compile()
    return nc


def kernel(batchs, label2embed):
    global LAST_RESULT
    from concourse.bass_utils import run_bass_kernel_spmd

    if "nc" not in _CACHE:
        _CACHE["nc"] = _build()
    nc = _CACHE["nc"]

    assert batchs.shape == (B, L, D) and label2embed.shape == (C, L, D)
    bf16 = ml_dtypes.bfloat16
    # K-major bf16, packed for 8KB/partition DMA rows:
    #   xt: [P, KT, B] (row p = k0-row-p ++ k1-row-p)
    #   wt: [2, P, KT, WH] (half h, row p = k0-cols ++ k1-cols)
    xtf = batchs.reshape(B, K).astype(bf16).T.reshape(KT, P, B)  # [KT, P, B]
    wtf = label2embed.reshape(C, K).astype(bf16).T.reshape(KT, P, C)
    xt = np.ascontiguousarray(xtf.transpose(1, 0, 2))  # [P, KT, B]
    wt = np.ascontiguousarray(
        np.stack(
            [wtf[:, :, h * WH : (h + 1) * WH].transpose(1, 0, 2) for h in range(2)]
        )
    )  # [2, P, KT, WH]
    in_maps = [
        {
            "x0": np.ascontiguousarray(xt[:, :, c * B_LOC : c * B_LOC + XD * P]),
            "xt": np.ascontiguousarray(xt[:, :, c * B_LOC : (c + 1) * B_LOC]),
            "wt": wt,
        }
        for c in range(N_CORES)
    ]
    res = run_bass_kernel_spmd(
        nc,
        in_maps,
        core_ids=list(range(N_CORES)),
        trace=PROFILE,
        trace_cores=list(range(N_CORES)) if (PROFILE and TRACE_ALL_CORES) else None,
    )
    LAST_RESULT = res
    return np.concatenate([r["out"] for r in res.results], axis=0).astype(np.float32)
